# revision 10
# baseline (speedup 1.0000x reference)
"""Trainium2 Bass kernel for a 3D attention block (v2: mh-outer + row-tiled S^T).

Reference computation (per batch b):
    xf = x[b].reshape(C, N)                       # C=256, N=4096
    q  = Wq @ xf + bq                             # [32, N]
    k  = Wk @ xf + bk                             # [32, N]
    v  = Wv @ xf + bv                             # [256, N]
    P  = softmax(q.T @ k, axis=-1)                # [N(m), N(n)]
    out[c, m] = sum_n v[c, n] * P[m, n]
    result = gamma * out + x[b]

Sharding: 8 cores = 2 batches x 4 chunks of 1024 query rows (m).
SPMD trick: every core receives x pre-rolled along n by -1024*j so its
query chunk sits at columns 0:1024.  Softmax rowsum and PV are
permutation-invariant in n, so k/v simply use the rolled order and no
per-core program differences are needed.

v2 structure (vs v1):
  - Outer loop over the two 512-column m-halves.  The PV accumulator
    shrinks to 2 PSUM banks, freeing 4 banks for a single [128, 2048]
    S^T staging tile (bufs=1 is enough: exp(ch) always completes
    during PV(ch-1), so S^T(ch+1) never stalls on the banks).
  - S^T is row-tiled 4-wide: the 4 k-tiles of a 512-n chunk are
    stacked on partition groups (k4[32*q4+c, ch, n']) and 4 K=32
    matmuls with tile_position=(32*q4, 0) run concurrently in the PE
    array (K=32 wastes 3/4 of the array otherwise; measured ~3x).
    q is replicated x4 on partitions via SBUF->SBUF DMAs so each row
    group streams its own moving operand.
  - The k projection writes the stacked layout directly with
    col-tiled matmuls (out partitions 32*q4).  The two K-halves of a
    group are emitted back-to-back (q0s,q0a,q1s,q1a,...): a start=True
    clears has_written for the whole bank, so every group's accumulate
    must land before the next group's start fires.
  - exp runs as ONE [128, 2048] ACT op per chunk-group, amortizing
    ScalarE per-op overhead (exp is the #2 engine at ~27us total).
  - Projections are emitted 2 chunks ahead of the attention loop and
    attention starts after chunk 0 (q(mh0) only needs x cols 0:512).
  - Rowsum per pass: 2 bf16 [128, 1024] accumulation chains on DVE
    (chain B's last chunk skips the q4=3 slice so the final rowsum
    matmul reads exp(ch7) directly), then 5 ginv^T matmuls accumulate
    in PSUM.  The rowsum comes out pre-divided by gamma.
  - v projection is one fp8e4 DoubleRow matmul per n-tile (validated
    in v1: rel ~1.2e-2 < 2e-2 gate); q/k stay fp16.

ATTN_KERNEL_REPEATS=<R> emits the body R times in one NEFF (timing via
slope; outputs are idempotent). ATTN_KERNEL_TRACE=1 captures an NTFF
profile via run_bass_kernel_spmd(trace=True).
ATTN_V_FP8=0 falls back to an fp16 v projection (2 matmuls/tile).
"""

import os

import numpy as np

import concourse.bass as bass
import concourse.mybir as mybir
import concourse.tile as tile
from concourse import bacc
from concourse.bass_utils import run_bass_kernel_spmd

F32 = mybir.dt.float32
F16 = mybir.dt.float16
BF16 = mybir.dt.bfloat16
F8 = mybir.dt.float8e4

C = 256
C8 = 32
N = 4096  # 16*16*16 voxels
MCHUNK = 1024  # query rows per core
MH = 512  # m-half processed per pass
NCH = 8  # 512-wide n chunks
NT = N // 128  # 32 key tiles
NCORES = 8
V_FP8 = int(os.environ.get("ATTN_V_FP8", "1"))

# info stashed by the last kernel() call (for test harnesses)
LAST_RESULTS = None


def _emit_body(nc, tc, io, rep):
    xf16, x8, wqk, wv8, bqk, bv, gamma, out = io
    r = f"_{rep}"
    with (
        tc.tile_pool(name="big" + r, bufs=1) as big,
        tc.tile_pool(name="ptp" + r, bufs=2) as ptp,
        tc.tile_pool(name="epi" + r, bufs=2) as epi,
        tc.tile_pool(name="pacc" + r, bufs=1, space="PSUM") as pacc,
        tc.tile_pool(name="pst4" + r, bufs=1, space="PSUM") as pst4,
        tc.tile_pool(name="psp" + r, bufs=2, space="PSUM") as psp,
    ):
        def chunk_sl(ch):
            return slice(ch * 512, (ch + 1) * 512)

        # ---- input DMAs.  Weights/constants on gpsimd; x chunks split
        # even/odd between the sync and vector queues so the stream
        # runs at 2-queue rate.  x8 pairs are enqueued lazily from
        # emit_proj so they never sit in front of critical transfers.
        wqk_t = big.tile([128, 2, 2 * C8], F16, name="wqk_t" + r)
        nc.gpsimd.dma_start(wqk_t[:], wqk[:])
        xf_t = big.tile([128, 2, N], F16, name="xf_t" + r)
        x8_t = None
        for ch in range(NCH):
            eng = nc.sync if ch % 2 == 0 else nc.scalar
            eng.dma_start(xf_t[:, :, chunk_sl(ch)], xf16[:, :, chunk_sl(ch)])
        if V_FP8:
            x8_t = big.tile([128, 2, N], F8, name="x8_t" + r)
            wv8_t = big.tile([128, 2, C], F8, name="wv8_t" + r)
            nc.gpsimd.dma_start(wv8_t[:], wv8[:])
        else:
            wv_t = big.tile([128, 2, C], F16, name="wv_t" + r)
            nc.gpsimd.dma_start(wv_t[:], wv8[:])

        # bqk comes pre-replicated x4 on partitions: [128, 2]
        bqk_t = big.tile([128, 2], F32, name="bqk_t" + r)
        nc.gpsimd.dma_start(bqk_t[:], bqk[:])
        bv_b = big.tile([128, C], F32, name="bv_b" + r)
        nc.gpsimd.dma_start(
            bv_b[:], bass.AP(tensor=bv, offset=0, ap=[[0, 128], [1, C]])
        )
        gamma_b = big.tile([128, 1], F32, name="gamma_b" + r)
        nc.gpsimd.dma_start(
            gamma_b[:], bass.AP(tensor=gamma, offset=0, ap=[[0, 128], [1, 1]])
        )
        # 1/gamma as the rowsum matmul's stationary vector: the rowsum
        # comes out pre-divided by gamma, so recip directly yields
        # gamma/rowsum and no separate gamma scaling stage is needed
        ginv_f = big.tile([128, 1], F32, name="ginv_f" + r)
        nc.vector.reciprocal(ginv_f[:], gamma_b[:])
        ginv_b = big.tile([128, 1], BF16, name="ginv_b" + r)
        nc.vector.tensor_copy(ginv_b[:], ginv_f[:])

        bq_t = bqk_t[0:C8, 0:1]
        bk4_t = bqk_t[:, 1:2]
        wu = big.tile([128, 512], BF16, name="wu" + r)
        nc.gpsimd.memset(wu[:], 0.0)
        ones_row = big.tile([1, 128], BF16, name="ones_row" + r)
        nc.gpsimd.memset(ones_row[:], 1.0)

        # q replicated x4 on partition groups, per m-half
        q4_sb = big.tile([128, 2, MH], F16, name="q4_sb" + r)
        # k tiles stacked x4 on partition groups: k4[32*q4+c, ch, n']
        k4_sb = big.tile([128, NCH, 128], F16, name="k4_sb" + r)
        vt_sb = big.tile([128, NT, C], BF16, name="vt_sb" + r)

        # PSUM: acc 2 banks, st4 4 banks, sp rotation 2 banks
        acc = [pacc.tile([128, MH], F32, name=f"acc{h}" + r) for h in range(2)]
        # rowsum accumulation chains (SBUF, bf16).  GPSIMD cannot read
        # PSUM, so it gets SBUF-only chain work: A = q4 0,1 ([128,1024]
        # DVE ops), C = q4 2 (DVE), D = q4 3 (GPSIMD, skips ch7).
        pacA = big.tile([128, 1024], BF16, name="pacA" + r)
        pacC = big.tile([128, 512], BF16, name="pacC" + r)
        pacD = big.tile([128, 512], BF16, name="pacD" + r)

        # warm-up matmuls on zeros: dependency-free filler the PE can
        # chew while the early DMAs land (keeps the HAM p-state up; acc
        # is reset by PV's start=True later).
        wcnt = [0]

        def emit_warm(n):
            for _ in range(n):
                nc.tensor.matmul(
                    acc[wcnt[0] % 2][:], wu[:, 0:128], wu[:],
                    start=True, stop=True,
                )
                wcnt[0] += 1

        emit_warm(6)

        def emit_qproj(mh):
            # q(mh) needs x columns mh*512:(mh+1)*512 == chunk mh
            q_ps = psp.tile([128, 512], F32, tag="sp", name=f"q_ps{mh}" + r)
            for kh in range(2):
                nc.tensor.matmul(
                    q_ps[:C8, :], wqk_t[:, kh, 0:C8], xf_t[:, kh, chunk_sl(mh)],
                    start=(kh == 0), stop=(kh == 1),
                )
            nc.vector.tensor_scalar_add(q4_sb[0:C8, mh, :], q_ps[:C8, :], bq_t)
            # replicate to partition groups 1..3 (different queues)
            for j, eng in ((1, nc.sync), (2, nc.scalar), (3, nc.gpsimd)):
                eng.dma_start(
                    q4_sb[32 * j : 32 * (j + 1), mh, :], q4_sb[0:C8, mh, :]
                )

        def emit_kproj(ch):
            # col-tiled into the stacked k4 layout: out partitions
            # 32*q4 hold the k values for n-tile 4*ch+q4.  K-half pairs
            # stay back-to-back per q4 (see module docstring).
            k_ps = psp.tile([128, 512], F32, tag="sp", name=f"k_ps{ch}" + r)
            for q4 in range(4):
                for kh in range(2):
                    nc.tensor.matmul(
                        k_ps[32 * q4 : 32 * (q4 + 1), 0:128],
                        wqk_t[:, kh, C8 : 2 * C8],
                        xf_t[:, kh, ch * 512 + 128 * q4 : ch * 512 + 128 * (q4 + 1)],
                        start=(kh == 0), stop=(kh == 1),
                        tile_position=(0, 32 * q4),
                    )
            nc.vector.tensor_scalar_add(k4_sb[:, ch, :], k_ps[:, 0:128], bk4_t)

        def emit_vproj(ch):
            for vh in range(2):
                v_ps = psp.tile(
                    [128, 512], F32, tag="sp", name=f"v_ps{ch}_{vh}" + r
                )
                for qq in range(2):
                    q4 = 2 * vh + qq
                    nt = 4 * ch + q4
                    ntsl = slice(nt * 128, (nt + 1) * 128)
                    csl = slice(qq * C, (qq + 1) * C)
                    if V_FP8:
                        nc.tensor.matmul(
                            v_ps[:, csl], x8_t[:, :, ntsl], wv8_t[:],
                            start=True, stop=True,
                            perf_mode=mybir.MatmulPerfMode.DoubleRow,
                        )
                    else:
                        nc.tensor.matmul(
                            v_ps[:, csl], xf_t[:, 0, ntsl], wv_t[:, 0, :],
                            start=True, stop=False,
                        )
                        nc.tensor.matmul(
                            v_ps[:, csl], xf_t[:, 1, ntsl], wv_t[:, 1, :],
                            start=False, stop=True,
                        )
                    # PSUM source: must be DVE (gpsimd has no PSUM port)
                    nc.vector.tensor_add(vt_sb[:, nt, :], v_ps[:, csl], bv_b[:])

        def emit_proj(ch):
            if V_FP8 and ch % 2 == 0:
                sl2 = slice(ch * 512, (ch + 2) * 512)
                nc.gpsimd.dma_start(x8_t[:, :, sl2], x8[:, :, sl2])
            emit_kproj(ch)
            emit_vproj(ch)

        # ---- main attention: two passes over m-halves ----
        for mh in range(2):
            pts = [None] * NCH

            def emit_st_exp(ch, mh=mh, pts=pts):
                st4 = pst4.tile(
                    [128, 2048], F32, tag="st", name=f"st{mh}_{ch}" + r
                )
                pt = ptp.tile(
                    [128, 2048], BF16, tag="pt", name=f"pt{mh}_{ch}" + r
                )
                for q4 in range(4):
                    psl = slice(32 * q4, 32 * (q4 + 1))
                    nc.tensor.matmul(
                        st4[:, q4 * 512 : (q4 + 1) * 512],
                        k4_sb[psl, ch, :],
                        q4_sb[psl, mh, :],
                        start=True, stop=True,
                        tile_position=(32 * q4, 0),
                    )
                nc.scalar.activation(
                    pt[:], st4[:], mybir.ActivationFunctionType.Exp
                )
                pts[ch] = pt

            def emit_pacc(ch, pts=pts):
                pt = pts[ch]
                chains = [
                    (nc.vector, pacA[:], slice(0, 1024)),
                    (nc.vector, pacC[:], slice(1024, 1536)),
                ]
                if ch < NCH - 1:  # q4=3 of ch7 feeds the rowsum directly
                    chains.append((nc.gpsimd, pacD[:], slice(1536, 2048)))
                for eng, dst, sl in chains:
                    if ch == 0:
                        eng.tensor_copy(dst, pt[:, sl])
                    else:
                        eng.tensor_add(dst, dst, pt[:, sl])

            def emit_pv(ch, pts=pts):
                pt = pts[ch]
                for q4 in range(4):
                    nt = 4 * ch + q4
                    first, last = nt == 0, nt == NT - 1
                    sl = slice(q4 * 512, (q4 + 1) * 512)
                    for h in range(2):
                        nc.tensor.matmul(
                            acc[h][:],
                            vt_sb[:, nt, h * 128 : (h + 1) * 128],
                            pt[:, sl],
                            start=first, stop=last,
                        )

            if mh == 0:
                emit_qproj(0)
                emit_proj(0)
                emit_proj(1)
                emit_qproj(1)

            for ch in range(NCH):
                if mh == 0 and ch < 2:
                    emit_warm(1)
                emit_st_exp(ch)
                if mh == 0 and ch < NCH - 2:
                    emit_proj(ch + 2)
                emit_pacc(ch)
                if ch >= 1:
                    emit_pv(ch - 1)
            emit_pv(NCH - 1)

            # ---- rowsum: rs[m] = ginv^T @ (sum of P^T chains) ----
            rs_tile = psp.tile([128, 512], F32, tag="sp", name=f"rs{mh}" + r)
            rs_ps = rs_tile[:1, :]
            nc.tensor.matmul(
                rs_ps[:], ginv_b[:], pts[NCH - 1][:, 3 * 512 : 4 * 512],
                start=True, stop=False,
            )
            rs_srcs = [pacA[:, 0:512], pacA[:, 512:1024], pacC[:], pacD[:]]
            for j, src in enumerate(rs_srcs):
                nc.tensor.matmul(
                    rs_ps[:], ginv_b[:], src, start=False, stop=(j == 3)
                )

            # ---- epilogue: out = acc * (gamma / rowsum) + x ----
            rinv = epi.tile([1, MH], F32, tag="rinv", name=f"rinv{mh}" + r)
            rinv_bf = epi.tile([1, MH], BF16, tag="rinvb", name=f"rinvb{mh}" + r)
            grecip_b = epi.tile([128, MH], F32, tag="grb", name=f"grb{mh}" + r)
            res = [
                epi.tile([128, MH], F16, tag=f"res{h}", name=f"res{h}_{mh}" + r)
                for h in range(2)
            ]
            nc.vector.reciprocal_approx_fast(rinv[:], rs_ps[:])
            nc.scalar.copy(rinv_bf[:], rinv[:])
            gr_tile = psp.tile([128, 512], F32, tag="sp", name=f"gr{mh}" + r)
            nc.tensor.matmul(
                gr_tile[:], ones_row[:], rinv_bf[:], start=True, stop=True
            )
            nc.scalar.copy(grecip_b[:], gr_tile[:])
            msl = slice(mh * 512, (mh + 1) * 512)
            for h in range(2):
                nc.vector.tensor_mul(res[h][:], acc[h][:], grecip_b[:])
            for h in range(2):
                # pass 0 overlaps pass 1, so its h=1 add can take the
                # slow gpsimd path; pass 1's adds stay on DVE (tail)
                eng = nc.vector if (h == 0 or mh == 1) else nc.gpsimd
                eng.tensor_add(res[h][:], res[h][:], xf_t[:, h, msl])
                nc.sync.dma_start(out[:, h, msl], res[h][:])


def _build(repeats=1):
    nc = bacc.Bacc("TRN2", target_bir_lowering=False, debug=False, num_devices=NCORES)

    xf16 = nc.dram_tensor("xf16", [128, 2, N], F16, kind="ExternalInput")
    x8 = nc.dram_tensor("x8", [128, 2, N], F8 if V_FP8 else F16, kind="ExternalInput")
    wqk = nc.dram_tensor("wqk", [128, 2, 2 * C8], F16, kind="ExternalInput")
    wv8 = nc.dram_tensor(
        "wv8", [128, 2, C], F8 if V_FP8 else F16, kind="ExternalInput"
    )
    bqk = nc.dram_tensor("bqk", [128, 2], F32, kind="ExternalInput")
    bv = nc.dram_tensor("bv", [1, C], F32, kind="ExternalInput")
    gamma = nc.dram_tensor("gamma", [1, 1], F32, kind="ExternalInput")
    out = nc.dram_tensor("out", [128, 2, MCHUNK], F16, kind="ExternalOutput")
    io = (xf16, x8, wqk, wv8, bqk, bv, gamma, out)

    with tile.TileContext(nc) as tc:
        for rep in range(repeats):
            _emit_body(nc, tc, io, rep)

    nc.compile()
    return nc


_NC_CACHE = {}


def _get_nc(repeats=1):
    if repeats not in _NC_CACHE:
        _NC_CACHE[repeats] = _build(repeats)
    return _NC_CACHE[repeats]


def _in_maps(x, Wq, bq, Wk, bk, Wv, bv, gamma):
    import ml_dtypes

    f8dt = ml_dtypes.float8_e4m3 if V_FP8 else np.float16
    xflat = x.reshape(2, C, N)
    # [C, N] -> [128, 2, N] with c = h*128 + p
    xh16 = xflat.astype(np.float16).reshape(2, 2, 128, N).transpose(0, 2, 1, 3)
    xh8 = xflat.astype(f8dt).reshape(2, 2, 128, N).transpose(0, 2, 1, 3)
    # [128, 2, 64]: wqk[p, h, o] = [Wq.T | Wk.T][h*128+p, o]
    wqk_full = np.concatenate([Wq.T, Wk.T], axis=1).astype(np.float16)  # [C, 64]
    wqk2 = np.ascontiguousarray(wqk_full.reshape(2, 128, 2 * C8).transpose(1, 0, 2))
    # [128, 2, 256]: wv8[p, h, co] = Wv[co, h*128+p]
    wv82 = np.ascontiguousarray(
        Wv.T.astype(f8dt).reshape(2, 128, C).transpose(1, 0, 2)
    )
    # [128, 2]: bqk replicated x4 on partitions
    bqk2 = np.stack([bq.reshape(C8), bk.reshape(C8)], axis=1).astype(np.float32)
    bqk4 = np.ascontiguousarray(np.tile(bqk2, (4, 1)))
    bv2 = np.ascontiguousarray(bv.reshape(1, C))
    g2 = np.ascontiguousarray(gamma.reshape(1, 1))

    maps = []
    for core in range(NCORES):
        b, j = core // 4, core % 4
        roll = -j * MCHUNK
        maps.append(
            {
                "xf16": np.ascontiguousarray(np.roll(xh16[b], roll, axis=2)),
                "x8": np.ascontiguousarray(np.roll(xh8[b], roll, axis=2)),
                "wqk": wqk2,
                "wv8": wv82,
                "bqk": bqk4,
                "bv": bv2,
                "gamma": g2,
            }
        )
    return maps


def kernel(x, Wq, bq, Wk, bk, Wv, bv, gamma):
    global LAST_RESULTS
    x = np.ascontiguousarray(np.asarray(x, dtype=np.float32))
    args = [np.asarray(a, dtype=np.float32) for a in (Wq, bq, Wk, bk, Wv, bv, gamma)]

    B, Cc, D, H, W = x.shape
    assert (B, Cc, D * H * W) == (2, C, N), x.shape

    repeats = int(os.environ.get("ATTN_KERNEL_REPEATS", "1"))
    nc = _get_nc(repeats)
    maps = _in_maps(x, *args)
    kwargs = {}
    if int(os.environ.get("ATTN_KERNEL_TRACE", "0")):
        kwargs = dict(
            trace=True,
            trace_cores=[0],
            tmpdir=os.environ.get("ATTN_KERNEL_TRACE_DIR"),
        )
    res = run_bass_kernel_spmd(nc, maps, core_ids=list(range(NCORES)), **kwargs)
    LAST_RESULTS = res

    outf = np.empty((B, C, N), dtype=np.float32)
    for core in range(NCORES):
        b, j = core // 4, core % 4
        o = np.asarray(res.results[core]["out"], dtype=np.float32)  # [128, 2, 1024]
        outf[b][:, j * MCHUNK : (j + 1) * MCHUNK] = o.transpose(1, 0, 2).reshape(
            C, MCHUNK
        )
    return outf.reshape(B, Cc, D, H, W)


# revision 21
# speedup vs baseline: 1.0469x; 1.0469x over previous
"""Trainium2 Bass kernel for a 3D attention block (v2: mh-outer + row-tiled S^T).

Reference computation (per batch b):
    xf = x[b].reshape(C, N)                       # C=256, N=4096
    q  = Wq @ xf + bq                             # [32, N]
    k  = Wk @ xf + bk                             # [32, N]
    v  = Wv @ xf + bv                             # [256, N]
    P  = softmax(q.T @ k, axis=-1)                # [N(m), N(n)]
    out[c, m] = sum_n v[c, n] * P[m, n]
    result = gamma * out + x[b]

Sharding: 8 cores = 2 batches x 4 chunks of 1024 query rows (m).
SPMD trick: every core receives x pre-rolled along n by -1024*j so its
query chunk sits at columns 0:1024.  Softmax rowsum and PV are
permutation-invariant in n, so k/v simply use the rolled order and no
per-core program differences are needed.

v2 structure (vs v1):
  - Outer loop over the two 512-column m-halves.  The PV accumulator
    shrinks to 2 PSUM banks, freeing 4 banks for a single [128, 2048]
    S^T staging tile (bufs=1 is enough: exp(ch) always completes
    during PV(ch-1), so S^T(ch+1) never stalls on the banks).
  - S^T is row-tiled 4-wide: the 4 k-tiles of a 512-n chunk are
    stacked on partition groups (k4[32*q4+c, ch, n']) and 4 K=32
    matmuls with tile_position=(32*q4, 0) run concurrently in the PE
    array (K=32 wastes 3/4 of the array otherwise; measured ~3x).
    q is replicated x4 on partitions via SBUF->SBUF DMAs so each row
    group streams its own moving operand.
  - The k projection writes the stacked layout directly with
    col-tiled matmuls (out partitions 32*q4).  The two K-halves of a
    group are emitted back-to-back (q0s,q0a,q1s,q1a,...): a start=True
    clears has_written for the whole bank, so every group's accumulate
    must land before the next group's start fires.
  - exp runs as ONE [128, 2048] ACT op per chunk-group, amortizing
    ScalarE per-op overhead (exp is the #2 engine at ~27us total).
  - Projections are emitted 2 chunks ahead of the attention loop and
    attention starts after chunk 0 (q(mh0) only needs x cols 0:512).
  - Rowsum per pass: 2 bf16 [128, 1024] accumulation chains on DVE
    (chain B's last chunk skips the q4=3 slice so the final rowsum
    matmul reads exp(ch7) directly), then 5 ginv^T matmuls accumulate
    in PSUM.  The rowsum comes out pre-divided by gamma.
  - v projection is one fp8e4 DoubleRow matmul per n-tile (validated
    in v1: rel ~1.2e-2 < 2e-2 gate); q/k stay fp16.

ATTN_KERNEL_REPEATS=<R> emits the body R times in one NEFF (timing via
slope; outputs are idempotent). ATTN_KERNEL_TRACE=1 captures an NTFF
profile via run_bass_kernel_spmd(trace=True).
ATTN_V_FP8=0 falls back to an fp16 v projection (2 matmuls/tile).
"""

import os

import numpy as np

import concourse.bass as bass
import concourse.mybir as mybir
import concourse.tile as tile
from concourse import bacc
from concourse.bass_utils import run_bass_kernel_spmd

F32 = mybir.dt.float32
F16 = mybir.dt.float16
BF16 = mybir.dt.bfloat16
F8 = mybir.dt.float8e4

C = 256
C8 = 32
N = 4096  # 16*16*16 voxels
MCHUNK = 1024  # query rows per core
MH = 512  # m-half processed per pass
NCH = 8  # 512-wide n chunks
NT = N // 128  # 32 key tiles
NCORES = 8
V_FP8 = int(os.environ.get("ATTN_V_FP8", "1"))

# info stashed by the last kernel() call (for test harnesses)
LAST_RESULTS = None


def _emit_body(nc, tc, io, rep):
    xf16, x8, wqk, wv8, bqk, bv, gamma, out = io
    r = f"_{rep}"
    with (
        tc.tile_pool(name="big" + r, bufs=1) as big,
        tc.tile_pool(name="ptp" + r, bufs=4) as ptp,
        tc.tile_pool(name="epi" + r, bufs=2) as epi,
        tc.tile_pool(name="pacc" + r, bufs=1, space="PSUM") as pacc,
        tc.tile_pool(name="pst4" + r, bufs=2, space="PSUM") as pst4,
    ):
        def chunk_sl(ch):
            return slice(ch * 512, (ch + 1) * 512)

        # ---- input DMAs.  Weights/constants on gpsimd; x chunks split
        # even/odd between the sync and vector queues so the stream
        # runs at 2-queue rate.  x8 pairs are enqueued lazily from
        # emit_proj so they never sit in front of critical transfers.
        # memsets first on gpsimd (its queue starts ~5.9us, before the
        # DMA gate) so the warm-up matmuls can begin immediately; the
        # dummy exp preloads the ACT table set (~2.7us) during the head
        wu = big.tile([128, 512], BF16, name="wu" + r)
        nc.gpsimd.memset(wu[:], 0.0)
        ones_row = big.tile([1, 128], BF16, name="ones_row" + r)
        nc.gpsimd.memset(ones_row[:], 1.0)
        scr1 = big.tile([1, 1], BF16, name="scr1" + r)
        nc.scalar.activation(
            scr1[:], wu[0:1, 0:1], mybir.ActivationFunctionType.Exp
        )

        wqk_t = big.tile([128, 2, 2 * C8], F16, name="wqk_t" + r)
        nc.gpsimd.dma_start(wqk_t[:], wqk[:])
        xf_t = big.tile([128, 2, N], F16, name="xf_t" + r)
        x8_t = None
        for ch in range(NCH):
            eng = nc.sync if ch % 2 == 0 else nc.scalar
            eng.dma_start(xf_t[:, :, chunk_sl(ch)], xf16[:, :, chunk_sl(ch)])
        if V_FP8:
            x8_t = big.tile([128, 2, N], F8, name="x8_t" + r)
            wv8_t = big.tile([128, 2, C], F8, name="wv8_t" + r)
            nc.gpsimd.dma_start(wv8_t[:], wv8[:])
        else:
            wv_t = big.tile([128, 2, C], F16, name="wv_t" + r)
            nc.gpsimd.dma_start(wv_t[:], wv8[:])

        # bqk comes pre-replicated x4 on partitions: [128, 2]
        bqk_t = big.tile([128, 2], F32, name="bqk_t" + r)
        nc.gpsimd.dma_start(bqk_t[:], bqk[:])
        bv_b = big.tile([128, C], F32, name="bv_b" + r)
        nc.gpsimd.dma_start(
            bv_b[:], bass.AP(tensor=bv, offset=0, ap=[[0, 128], [1, C]])
        )
        gamma_b = big.tile([128, 1], F32, name="gamma_b" + r)
        nc.gpsimd.dma_start(
            gamma_b[:], bass.AP(tensor=gamma, offset=0, ap=[[0, 128], [1, 1]])
        )
        # 1/gamma as the rowsum matmul's stationary vector: the rowsum
        # comes out pre-divided by gamma, so recip directly yields
        # gamma/rowsum and no separate gamma scaling stage is needed
        ginv_f = big.tile([128, 1], F32, name="ginv_f" + r)
        nc.vector.reciprocal(ginv_f[:], gamma_b[:])
        ginv_b = big.tile([128, 1], BF16, name="ginv_b" + r)
        nc.vector.tensor_copy(ginv_b[:], ginv_f[:])

        bq_t = bqk_t[0:C8, 0:1]
        bk4_t = bqk_t[:, 1:2]

        # q replicated x4 on partition groups, per m-half
        q4_sb = big.tile([128, 2, MH], F16, name="q4_sb" + r)
        # k tiles stacked x4 on partition groups: k4[32*q4+c, ch, n']
        k4_sb = big.tile([128, NCH, 128], F16, name="k4_sb" + r)
        vt_sb = big.tile([128, NT, C], BF16, name="vt_sb" + r)

        # PSUM: 4 acc banks (2 per pass; pass-1's double as pass-0's
        # projection scratch) + 4 staging banks (2 tiles, bufs=2).
        # Separate per-pass acc banks let pass-1's PV start without
        # waiting for pass-0's epilogue to drain its accumulator.
        accp = [
            [pacc.tile([128, MH], F32, name=f"acc{p}{h}" + r) for h in range(2)]
            for p in range(2)
        ]
        scr_cnt = [0]

        def scratch_ps():
            # pass-0 psum scratch rotates over pass-1's (not yet used)
            # acc banks; the tile tracker serializes reuse via the DVE
            # drains.  rs/gr of pass p use pass (1-p)'s banks.
            t = accp[1][scr_cnt[0] % 2]
            scr_cnt[0] += 1
            return t
        # rowsum accumulation chains (SBUF, bf16).  GPSIMD cannot read
        # PSUM, so it gets SBUF-only chain work: A = q4 0,1 ([128,1024]
        # DVE ops), C = q4 2 (DVE), D = q4 3 (GPSIMD, skips ch7).
        pacA = big.tile([128, 1024], BF16, name="pacA" + r)
        pacC = big.tile([128, 512], BF16, name="pacC" + r)
        pacD = big.tile([128, 512], BF16, name="pacD" + r)

        # warm-up matmuls on zeros: dependency-free filler the PE can
        # chew while the early DMAs land (keeps the HAM p-state up; acc
        # is reset by PV's start=True later).
        wcnt = [0]

        def emit_warm(n):
            for _ in range(n):
                nc.tensor.matmul(
                    accp[0][wcnt[0] % 2][:], wu[:, 0:128], wu[:],
                    start=True, stop=True,
                )
                wcnt[0] += 1

        emit_warm(6)

        def emit_qproj(mh):
            # q(mh) needs x columns mh*512:(mh+1)*512 == chunk mh
            q_ps = scratch_ps()
            for kh in range(2):
                nc.tensor.matmul(
                    q_ps[:C8, :], wqk_t[:, kh, 0:C8], xf_t[:, kh, chunk_sl(mh)],
                    start=(kh == 0), stop=(kh == 1),
                )
            nc.vector.tensor_scalar_add(q4_sb[0:C8, mh, :], q_ps[:C8, :], bq_t)
            # replicate to partition groups 1..3 (different queues)
            for j, eng in ((1, nc.sync), (2, nc.scalar), (3, nc.gpsimd)):
                eng.dma_start(
                    q4_sb[32 * j : 32 * (j + 1), mh, :], q4_sb[0:C8, mh, :]
                )

        def emit_kproj(ch):
            # col-tiled into the stacked k4 layout: out partitions
            # 32*q4 hold the k values for n-tile 4*ch+q4.  K-half pairs
            # stay back-to-back per q4 (see module docstring).
            k_ps = scratch_ps()
            for q4 in range(4):
                for kh in range(2):
                    nc.tensor.matmul(
                        k_ps[32 * q4 : 32 * (q4 + 1), 0:128],
                        wqk_t[:, kh, C8 : 2 * C8],
                        xf_t[:, kh, ch * 512 + 128 * q4 : ch * 512 + 128 * (q4 + 1)],
                        start=(kh == 0), stop=(kh == 1),
                        tile_position=(0, 32 * q4),
                    )
            nc.vector.tensor_scalar_add(k4_sb[:, ch, :], k_ps[:, 0:128], bk4_t)

        def emit_vproj(ch):
            for vh in range(2):
                v_ps = scratch_ps()
                for qq in range(2):
                    q4 = 2 * vh + qq
                    nt = 4 * ch + q4
                    ntsl = slice(nt * 128, (nt + 1) * 128)
                    csl = slice(qq * C, (qq + 1) * C)
                    if V_FP8:
                        nc.tensor.matmul(
                            v_ps[:, csl], x8_t[:, :, ntsl], wv8_t[:],
                            start=True, stop=True,
                            perf_mode=mybir.MatmulPerfMode.DoubleRow,
                        )
                    else:
                        nc.tensor.matmul(
                            v_ps[:, csl], xf_t[:, 0, ntsl], wv_t[:, 0, :],
                            start=True, stop=False,
                        )
                        nc.tensor.matmul(
                            v_ps[:, csl], xf_t[:, 1, ntsl], wv_t[:, 1, :],
                            start=False, stop=True,
                        )
                    # PSUM source: must be DVE (gpsimd has no PSUM port)
                    nc.vector.tensor_add(vt_sb[:, nt, :], v_ps[:, csl], bv_b[:])

        def emit_proj(ch):
            if V_FP8 and ch % 2 == 0:
                sl2 = slice(ch * 512, (ch + 2) * 512)
                nc.gpsimd.dma_start(x8_t[:, :, sl2], x8[:, :, sl2])
            emit_kproj(ch)
            emit_vproj(ch)

        # ---- main attention: two passes over m-halves ----
        for mh in range(2):
            pts = [None] * NCH  # per chunk: (ptA, ptB) for q4 01 / 23

            def emit_st_exp(ch, mh=mh, pts=pts):
                # two 2-wide row-tiled groups; exp of group g starts
                # while group g+1's matmuls still stream
                pt_pair = []
                for g in range(2):
                    st = pst4.tile(
                        [128, 1024], F32, tag="st", name=f"st{mh}_{ch}_{g}" + r
                    )
                    pt = ptp.tile(
                        [128, 1024], BF16, tag="pt", name=f"pt{mh}_{ch}_{g}" + r
                    )
                    for qq in range(2):
                        q4 = 2 * g + qq
                        psl = slice(32 * q4, 32 * (q4 + 1))
                        nc.tensor.matmul(
                            st[:, qq * 512 : (qq + 1) * 512],
                            k4_sb[psl, ch, :],
                            q4_sb[psl, mh, :],
                            start=True, stop=True,
                            tile_position=(32 * q4, 0),
                        )
                    nc.scalar.activation(
                        pt[:], st[:], mybir.ActivationFunctionType.Exp
                    )
                    pt_pair.append(pt)
                pts[ch] = pt_pair

            def emit_pacc(ch, pts=pts):
                ptA, ptB = pts[ch]
                chains = [
                    (nc.vector, pacA[:], ptA[:]),
                    (nc.vector, pacC[:], ptB[:, 0:512]),
                ]
                if ch < NCH - 1:  # q4=3 of ch7 feeds the rowsum directly
                    chains.append((nc.gpsimd, pacD[:], ptB[:, 512:1024]))
                for eng, dst, src in chains:
                    if ch == 0:
                        eng.tensor_copy(dst, src)
                    else:
                        eng.tensor_add(dst, dst, src)

            def emit_pv(ch, mh=mh, pts=pts):
                ptA, ptB = pts[ch]
                for q4 in range(4):
                    nt = 4 * ch + q4
                    first, last = nt == 0, nt == NT - 1
                    pt = ptA if q4 < 2 else ptB
                    sl = slice((q4 % 2) * 512, (q4 % 2 + 1) * 512)
                    for h in range(2):
                        nc.tensor.matmul(
                            accp[mh][h][:],
                            vt_sb[:, nt, h * 128 : (h + 1) * 128],
                            pt[:, sl],
                            start=first, stop=last,
                        )

            if mh == 0:
                emit_qproj(0)
                emit_proj(0)
                emit_proj(1)
                emit_qproj(1)

            for ch in range(NCH):
                if mh == 0 and ch < 2:
                    emit_warm(1)
                emit_st_exp(ch)
                if mh == 0 and ch < NCH - 2:
                    emit_proj(ch + 2)
                emit_pacc(ch)
                if ch >= 1:
                    emit_pv(ch - 1)
            emit_pv(NCH - 1)

            # ---- rowsum: rs[m] = ginv^T @ (sum of P^T chains) ----
            # rs/gr live in the OTHER pass's acc banks (free by now)
            rs_tile = accp[1 - mh][0]
            rs_ps = rs_tile[:1, :]
            nc.tensor.matmul(
                rs_ps[:], ginv_b[:], pts[NCH - 1][1][:, 512:1024],
                start=True, stop=False,
            )
            rs_srcs = [pacA[:, 0:512], pacA[:, 512:1024], pacC[:], pacD[:]]
            for j, src in enumerate(rs_srcs):
                nc.tensor.matmul(
                    rs_ps[:], ginv_b[:], src, start=False, stop=(j == 3)
                )

            # ---- epilogue: out = acc * (gamma / rowsum) + x ----
            rinv = epi.tile([1, MH], F32, tag="rinv", name=f"rinv{mh}" + r)
            rinv_bf = epi.tile([1, MH], BF16, tag="rinvb", name=f"rinvb{mh}" + r)
            grecip_b = epi.tile([128, MH], F32, tag="grb", name=f"grb{mh}" + r)
            res = [
                epi.tile([128, MH], F16, tag=f"res{h}", name=f"res{h}_{mh}" + r)
                for h in range(2)
            ]
            nc.vector.reciprocal_approx_fast(rinv[:], rs_ps[:])
            nc.scalar.copy(rinv_bf[:], rinv[:])
            gr_tile = accp[1 - mh][1]
            nc.tensor.matmul(
                gr_tile[:], ones_row[:], rinv_bf[:], start=True, stop=True
            )
            nc.scalar.copy(grecip_b[:], gr_tile[:])
            msl = slice(mh * 512, (mh + 1) * 512)
            for h in range(2):
                nc.vector.tensor_mul(res[h][:], accp[mh][h][:], grecip_b[:])
            for h in range(2):
                # pass 0 overlaps pass 1, so its h=1 add can take the
                # slow gpsimd path; pass 1's adds stay on DVE (tail)
                eng = nc.vector if (h == 0 or mh == 1) else nc.gpsimd
                eng.tensor_add(res[h][:], res[h][:], xf_t[:, h, msl])
                nc.sync.dma_start(out[:, h, msl], res[h][:])


def _build(repeats=1):
    nc = bacc.Bacc("TRN2", target_bir_lowering=False, debug=False, num_devices=NCORES)

    xf16 = nc.dram_tensor("xf16", [128, 2, N], F16, kind="ExternalInput")
    x8 = nc.dram_tensor("x8", [128, 2, N], F8 if V_FP8 else F16, kind="ExternalInput")
    wqk = nc.dram_tensor("wqk", [128, 2, 2 * C8], F16, kind="ExternalInput")
    wv8 = nc.dram_tensor(
        "wv8", [128, 2, C], F8 if V_FP8 else F16, kind="ExternalInput"
    )
    bqk = nc.dram_tensor("bqk", [128, 2], F32, kind="ExternalInput")
    bv = nc.dram_tensor("bv", [1, C], F32, kind="ExternalInput")
    gamma = nc.dram_tensor("gamma", [1, 1], F32, kind="ExternalInput")
    out = nc.dram_tensor("out", [128, 2, MCHUNK], F16, kind="ExternalOutput")
    io = (xf16, x8, wqk, wv8, bqk, bv, gamma, out)

    with tile.TileContext(nc) as tc:
        for rep in range(repeats):
            _emit_body(nc, tc, io, rep)

    nc.compile()
    return nc


_NC_CACHE = {}


def _get_nc(repeats=1):
    if repeats not in _NC_CACHE:
        _NC_CACHE[repeats] = _build(repeats)
    return _NC_CACHE[repeats]


def _in_maps(x, Wq, bq, Wk, bk, Wv, bv, gamma):
    import ml_dtypes

    f8dt = ml_dtypes.float8_e4m3 if V_FP8 else np.float16
    xflat = x.reshape(2, C, N)
    # [C, N] -> [128, 2, N] with c = h*128 + p
    xh16 = xflat.astype(np.float16).reshape(2, 2, 128, N).transpose(0, 2, 1, 3)
    xh8 = xflat.astype(f8dt).reshape(2, 2, 128, N).transpose(0, 2, 1, 3)
    # [128, 2, 64]: wqk[p, h, o] = [Wq.T | Wk.T][h*128+p, o]
    wqk_full = np.concatenate([Wq.T, Wk.T], axis=1).astype(np.float16)  # [C, 64]
    wqk2 = np.ascontiguousarray(wqk_full.reshape(2, 128, 2 * C8).transpose(1, 0, 2))
    # [128, 2, 256]: wv8[p, h, co] = Wv[co, h*128+p]
    wv82 = np.ascontiguousarray(
        Wv.T.astype(f8dt).reshape(2, 128, C).transpose(1, 0, 2)
    )
    # [128, 2]: bqk replicated x4 on partitions
    bqk2 = np.stack([bq.reshape(C8), bk.reshape(C8)], axis=1).astype(np.float32)
    bqk4 = np.ascontiguousarray(np.tile(bqk2, (4, 1)))
    bv2 = np.ascontiguousarray(bv.reshape(1, C))
    g2 = np.ascontiguousarray(gamma.reshape(1, 1))

    maps = []
    for core in range(NCORES):
        b, j = core // 4, core % 4
        roll = -j * MCHUNK
        maps.append(
            {
                "xf16": np.ascontiguousarray(np.roll(xh16[b], roll, axis=2)),
                "x8": np.ascontiguousarray(np.roll(xh8[b], roll, axis=2)),
                "wqk": wqk2,
                "wv8": wv82,
                "bqk": bqk4,
                "bv": bv2,
                "gamma": g2,
            }
        )
    return maps


def kernel(x, Wq, bq, Wk, bk, Wv, bv, gamma):
    global LAST_RESULTS
    x = np.ascontiguousarray(np.asarray(x, dtype=np.float32))
    args = [np.asarray(a, dtype=np.float32) for a in (Wq, bq, Wk, bk, Wv, bv, gamma)]

    B, Cc, D, H, W = x.shape
    assert (B, Cc, D * H * W) == (2, C, N), x.shape

    repeats = int(os.environ.get("ATTN_KERNEL_REPEATS", "1"))
    nc = _get_nc(repeats)
    maps = _in_maps(x, *args)
    kwargs = {}
    if int(os.environ.get("ATTN_KERNEL_TRACE", "0")):
        kwargs = dict(
            trace=True,
            trace_cores=[0],
            tmpdir=os.environ.get("ATTN_KERNEL_TRACE_DIR"),
        )
    res = run_bass_kernel_spmd(nc, maps, core_ids=list(range(NCORES)), **kwargs)
    LAST_RESULTS = res

    outf = np.empty((B, C, N), dtype=np.float32)
    for core in range(NCORES):
        b, j = core // 4, core % 4
        o = np.asarray(res.results[core]["out"], dtype=np.float32)  # [128, 2, 1024]
        outf[b][:, j * MCHUNK : (j + 1) * MCHUNK] = o.transpose(1, 0, 2).reshape(
            C, MCHUNK
        )
    return outf.reshape(B, Cc, D, H, W)
